# revision 1
# baseline (speedup 1.0000x reference)
"""Trainium2 Bass kernel for nn_AttentionBlock (pre-LN causal attention + SiLU MLP).

8-core SPMD strategy (data-parallel over batch x sequence-parallel over rows):
  - core c handles sample b = c // NPOS, position g = c % NPOS
  - the L rows of a sample are split into NBLK blocks of BS rows; each core owns
    NBPC blocks, paired to balance causal-attention cost (host-chosen pairing)
  - every core computes LN1 + K^T/V for the full sample (replicated), q/proj/MLP
    only for its own rows.  Per-core differences in causal trip counts are
    handled with tc.If branches on partition_id; per-core data differences
    (mask_len row selection) are handled via input data (sel blend / vbar).

All matmul layouts are "transposed" (feature dim on partitions) so no on-device
transposes are needed anywhere; the host feeds x pre-transposed and re-assembles
the transposed output.
"""
import math
from contextlib import ExitStack
from dataclasses import dataclass

import ml_dtypes
import numpy as np

import concourse.bass as bass
import concourse.mybir as mybir
import concourse.tile as tile
from concourse import bacc
from concourse.bass import ds, ts
from concourse.bass_utils import run_bass_kernel_spmd

F32 = mybir.dt.float32
BF16 = mybir.dt.bfloat16
AF = mybir.ActivationFunctionType
ALU = mybir.AluOpType
BF16NP = ml_dtypes.bfloat16


@dataclass
class Cfg:
    B: int = 2
    L: int = 2048
    E: int = 768
    H: int = 12
    D: int = 64
    FF: int = 3072
    BS: int = 256          # query block rows
    n_cores: int = 8
    eps: float = 1e-6

    @property
    def NPOS(self):
        return self.n_cores // self.B

    @property
    def NBLK(self):
        return self.L // self.BS

    @property
    def NBPC(self):
        return self.NBLK // self.NPOS   # blocks per core

    @property
    def R(self):
        return self.NBPC * self.BS      # own rows per core

    @property
    def EC(self):
        return self.E // 128

    @property
    def FC(self):
        return self.FF // 128

    @property
    def LC(self):
        return self.L // 128

    @property
    def HC(self):
        return self.H // 2              # head-pair chunks (= EC since E = H*D, D=64)


def plan_blocks(cfg: Cfg, mask_lens):
    """Choose jmax (number of attention-active blocks) and block pairing."""
    mmax = int(max(int(m) for m in mask_lens))
    mmax = max(1, min(cfg.L, mmax))
    jmax = (mmax + cfg.BS - 1) // cfg.BS          # blocks [0, jmax) need causal attn
    def cost(j):
        return (j + 1) if j < jmax else 0
    order = sorted(range(cfg.NBLK), key=lambda j: -cost(j))
    pairs = []
    for g in range(cfg.NPOS):
        blocks = []
        for s in range(cfg.NBPC):
            # snake over sorted order: pair heavy with light
            idx = g if s % 2 == 0 else (cfg.NBLK - 1 - g)
            blocks.append(order[idx])
        pairs.append(tuple(blocks))
    return pairs, jmax


def kc_of(cfg: Cfg, j, jmax):
    """number of 128-wide key chunks block j attends to (0 if mask-free)."""
    if j >= jmax:
        return 0
    return (j + 1) * cfg.BS // 128


# ----------------------------------------------------------------------------
# program builder
# ----------------------------------------------------------------------------

def build_program(cfg: Cfg, pairs, jmax, flags, bake_g=None, stage_limit=99, repeat=1,
                  loop_n=1, ablate=()):
    """flags: dict with bools: bq, bk, bv, bproj, bfc, bout, ln1aff, ln2aff

    bake_g: if set, emit only that variant's attention without tc.If (for
    timing estimation with TimelineSim)."""
    E, L, H, D2, FF, BS, R = cfg.E, cfg.L, cfg.H, cfg.D, cfg.FF, cfg.BS, cfg.R
    EC, FC, LC, HC, NBPC = cfg.EC, cfg.FC, cfg.LC, cfg.HC, cfg.NBPC
    KEYS = jmax * BS
    KC = KEYS // 128
    qscale = 1.0 / math.sqrt(cfg.D)

    nc = bacc.Bacc(num_devices=cfg.n_cores)

    # ---- dram I/O ----
    d_xTf = nc.dram_tensor("xT_full", [128, EC * L], BF16, kind="ExternalInput")
    d_xTo = nc.dram_tensor("xT_own", [128, EC * R], F32, kind="ExternalInput")
    d_wq = nc.dram_tensor("wq", [128, EC * E], BF16, kind="ExternalInput")
    d_wk = nc.dram_tensor("wk", [128, EC * E], BF16, kind="ExternalInput")
    d_wv = nc.dram_tensor("wv", [128, EC * E], BF16, kind="ExternalInput")
    d_wp = nc.dram_tensor("wproj", [128, EC * E], BF16, kind="ExternalInput")
    d_wfc = nc.dram_tensor("wfc", [FC, 128, EC * 128], BF16, kind="ExternalInput")
    d_wout = nc.dram_tensor("wout", [EC, 128, FC * 128], BF16, kind="ExternalInput")
    d_bq = nc.dram_tensor("bq", [128, EC], F32, kind="ExternalInput")
    d_bk = nc.dram_tensor("bk", [128, EC], F32, kind="ExternalInput")
    d_bv = nc.dram_tensor("bv", [1, E], BF16, kind="ExternalInput")
    d_bp = nc.dram_tensor("bproj", [128, EC], F32, kind="ExternalInput")
    d_bfc = nc.dram_tensor("bfc", [128, FC], F32, kind="ExternalInput")
    d_bout = nc.dram_tensor("bout", [128, EC], F32, kind="ExternalInput")
    d_ln = nc.dram_tensor("lnp", [128, 4, EC], F32, kind="ExternalInput")  # g1,b1,g2,b2
    d_selb = nc.dram_tensor("selb", [128, R], BF16, kind="ExternalInput")
    d_masks = nc.dram_tensor("diagmasks", [2, 128, BS], BF16, kind="ExternalInput")
    d_out = nc.dram_tensor("outT", [128, EC * R], F32, kind="ExternalOutput")

    with tile.TileContext(nc) as tc, ExitStack() as st:
        # ------- L0: persistent pools -------
        cpool = st.enter_context(tc.tile_pool(name="consts", bufs=1))

        wq_s = cpool.tile([128, EC, E], BF16)
        wk_s = cpool.tile([128, EC, E], BF16)
        wv_s = cpool.tile([128, EC, E], BF16)
        wp_s = cpool.tile([128, EC, E], BF16)
        xo_s = cpool.tile([128, EC, R], F32)
        bq_s = cpool.tile([128, EC], F32)
        bk_s = cpool.tile([128, EC], F32)
        bv_s = cpool.tile([1, E], BF16)
        bp_s = cpool.tile([128, EC], F32)
        bfc_s = cpool.tile([128, FC], F32)
        bout_s = cpool.tile([128, EC], F32)
        ln_s = cpool.tile([128, 4, EC], F32)
        selb_s = cpool.tile([128, R], BF16)
        maskA = cpool.tile([128, BS], BF16)
        maskB = cpool.tile([128, BS], BF16)
        ones_col = cpool.tile([128, 1], BF16)
        ones_row = cpool.tile([1, 128], BF16)
        ones_rf = cpool.tile([1, 64], F32)
        ones_11 = cpool.tile([1, 1], BF16)
        eps_11 = cpool.tile([1, 1], F32)
        yT = cpool.tile([128, HC, R], BF16)
        vbarT = cpool.tile([128, EC, 1], F32)
        vrow = cpool.tile([1, E], BF16)

      # loop body emitted `repeat` times (timing calibration); closures below
      # reference gpsum, set per-iteration.
      # NOTE: 6-space indent keeps the original body indentation valid.

        def emit_body(ri):
          def pbcast(out_ap, in_ap, ch):
              if "nobc" in ablate:
                  nc.vector.memset(out_ap, 1.0)
              else:
                  nc.gpsimd.partition_broadcast(out_ap, in_ap, channels=ch)
          with tc.tile_pool(name=f"gpsum{ri}", bufs=2, space="PSUM") as gpsum:
            nc.sync.dma_start(wq_s[:], d_wq.rearrange("p (c n) -> p c n", c=EC))
            nc.sync.dma_start(wk_s[:], d_wk.rearrange("p (c n) -> p c n", c=EC))
            nc.sync.dma_start(wv_s[:], d_wv.rearrange("p (c n) -> p c n", c=EC))
            nc.sync.dma_start(wp_s[:], d_wp.rearrange("p (c n) -> p c n", c=EC))
            nc.sync.dma_start(xo_s[:], d_xTo.rearrange("p (c n) -> p c n", c=EC))
            nc.sync.dma_start(bq_s[:], d_bq[:])
            nc.sync.dma_start(bk_s[:], d_bk[:])
            nc.sync.dma_start(bv_s[:], d_bv[:])
            nc.sync.dma_start(bp_s[:], d_bp[:])
            nc.sync.dma_start(bfc_s[:], d_bfc[:])
            nc.sync.dma_start(bout_s[:], d_bout[:])
            nc.sync.dma_start(ln_s[:], d_ln[:])
            nc.sync.dma_start(selb_s[:], d_selb[:])
            nc.sync.dma_start(maskA[:], d_masks[0])
            nc.sync.dma_start(maskB[:], d_masks[1])
            nc.vector.memset(ones_col[:], 1.0)
            nc.vector.memset(ones_row[:], 1.0)
            nc.vector.memset(ones_rf[:], 1.0)
            nc.vector.memset(ones_11[:], 1.0)
            nc.vector.memset(eps_11[:], cfg.eps)
            nc.vector.memset(yT[:], 0.0)
            # ============================================================
            # helper: layernorm in transposed layout
            # x_bf: sbuf [128, EC, N] bf16 ; writes zT [128, EC, N] bf16
            # ============================================================
            def ln_transposed(pool, x_bf, N, gb_idx, zT_out, tag, bpool):
                """zT_out may alias x_bf (in-place LN apply)."""
                gi, bi = gb_idx
                affine = flags["ln1aff"] if gb_idx == (0, 1) else flags["ln2aff"]
                for cg0 in range(0, N, 512):
                    w = min(512, N - cg0)
                    ps_su = gpsum.tile([1, 512], F32, tag="gp", name=f"pssu{tag}{cg0}")
                    ps_sq = gpsum.tile([1, 512], F32, tag="gp", name=f"pssq{tag}{cg0}")
                    for c in range(EC):
                        nc.tensor.matmul(ps_su[:, :w], ones_col[:], x_bf[:, c, cg0:cg0 + w],
                                         start=(c == 0), stop=(c == EC - 1))
                    for c in range(EC):
                        sq = pool.tile([128, 512], BF16, tag="lnsq", name=f"sq{tag}{cg0}{c}")
                        nc.vector.tensor_tensor(sq[:, :w], x_bf[:, c, cg0:cg0 + w],
                                                x_bf[:, c, cg0:cg0 + w], ALU.mult)
                        nc.tensor.matmul(ps_sq[:, :w], ones_col[:], sq[:, :w],
                                         start=(c == 0), stop=(c == EC - 1))
                    # mu = sum/E ; var = sumsq/E - mu^2 ; a = 1/sqrt(var+eps) ; b = -mu*a
                    mu = pool.tile([1, 512], F32, tag="lnmu", name=f"mu{tag}{cg0}")
                    nc.vector.tensor_scalar_mul(mu[:, :w], ps_su[:, :w], 1.0 / E)
                    va = pool.tile([1, 512], F32, tag="lnva", name=f"va{tag}{cg0}")
                    nc.vector.tensor_scalar_mul(va[:, :w], ps_sq[:, :w], 1.0 / E)
                    t1 = pool.tile([1, 512], F32, tag="lnt1", name=f"t1{tag}{cg0}")
                    nc.vector.tensor_tensor(t1[:, :w], mu[:, :w], mu[:, :w], ALU.mult)
                    nc.vector.tensor_sub(va[:, :w], va[:, :w], t1[:, :w])
                    nc.scalar.activation(t1[:, :w], va[:, :w], AF.Sqrt, bias=eps_11[:])
                    nc.vector.reciprocal(va[:, :w], t1[:, :w])     # va = rstd = a
                    arow = pool.tile([1, 512], BF16, tag="lnar", name=f"ar{tag}{cg0}")
                    nc.vector.tensor_copy(arow[:, :w], va[:, :w])
                    nc.vector.tensor_tensor(t1[:, :w], mu[:, :w], va[:, :w], ALU.mult)
                    brow = pool.tile([1, 512], BF16, tag="lnbr", name=f"br{tag}{cg0}")
                    nc.vector.tensor_scalar_mul(brow[:, :w], t1[:, :w], -1.0)
                    ab = bpool.tile([128, 2, 512], F32, tag="lnab", name=f"ab{tag}{cg0}")
                    nc.tensor.matmul(ab[:, 0, :w], ones_row[:], arow[:, :w],
                                     start=True, stop=True)
                    nc.tensor.matmul(ab[:, 1, :w], ones_row[:], brow[:, :w],
                                     start=True, stop=True)
                    for c in range(EC):
                        nc.vector.tensor_tensor(zT_out[:, c, cg0:cg0 + w],
                                                x_bf[:, c, cg0:cg0 + w], ab[:, 0, :w], ALU.mult)
                        nc.vector.tensor_tensor(zT_out[:, c, cg0:cg0 + w],
                                                zT_out[:, c, cg0:cg0 + w], ab[:, 1, :w], ALU.add)
                        if affine:
                            nc.vector.tensor_scalar(zT_out[:, c, cg0:cg0 + w],
                                                    zT_out[:, c, cg0:cg0 + w],
                                                    ln_s[:, gi, c:c + 1], ln_s[:, bi, c:c + 1],
                                                    ALU.mult, ALU.add)

            # ------- L2: sample-wide tensors (die after attention) -------
            with tc.tile_pool(name="l2", bufs=1) as l2:
                # zT / zqT are computed in place over the loaded x tiles
                zT = l2.tile([128, EC, L], BF16, tag="zT", name="zT")
                nc.sync.dma_start(zT[:], d_xTf.rearrange("p (c n) -> p c n", c=EC))
                zqT = l2.tile([128, EC, R], BF16, tag="zqT", name="zqT")
                nc.vector.tensor_copy(zqT[:], xo_s[:])
                qTs = l2.tile([128, HC, R], BF16, tag="qTs", name="qTs")
                kTs = l2.tile([128, HC, KEYS], BF16, tag="kTs", name="kTs")
                Vs = l2.tile([128, KC, H, 65], BF16, tag="Vs", name="Vs")

                # ------- L3: LN1 scratch (dies after zT/zqT written) -------
                if stage_limit >= 1:
                    with tc.tile_pool(name="l3", bufs=2) as l3, \
                         tc.tile_pool(name=f"bp1{ri}", bufs=2, space="PSUM") as bp1:
                        ln_transposed(l3, zT, L, (0, 1), zT, "f", bp1)
                        ln_transposed(l3, zqT, R, (0, 1), zqT, "o", bp1)

                # ------- QKV -------
                # q^T (own rows): [128(hd), HC, R]
                for m in range(EC if stage_limit >= 2 else 0):
                    ps = gpsum.tile([128, 512], F32, tag="gp", name=f"psq{m}")
                    for c in range(EC):
                        nc.tensor.matmul(ps[:, :R], wq_s[:, c, ts(m, 128)], zqT[:, c, :],
                                         start=(c == 0), stop=(c == EC - 1))
                    if flags["bq"]:
                        nc.vector.tensor_scalar(qTs[:, m, :], ps[:, :R], bq_s[:, m:m + 1],
                                                qscale, ALU.add, ALU.mult)
                    else:
                        nc.vector.tensor_scalar_mul(qTs[:, m, :], ps[:, :R], qscale)
                # k^T (keys 0..keys_g): [128(hd), HC, KEYS] — emitted per variant
                # so each core only computes the key range its blocks reach
                def emit_kT(keys_g, gtag):
                    if stage_limit < 2:
                        return
                    for m in range(EC):
                        for n0 in range(0, keys_g, 512):
                            w = min(512, keys_g - n0)
                            ps = gpsum.tile([128, 512], F32, tag="gp",
                                            name=f"psk{gtag}{m}{n0}")
                            for c in range(EC):
                                nc.tensor.matmul(ps[:, :w], wk_s[:, c, ts(m, 128)],
                                                 zT[:, c, n0:n0 + w],
                                                 start=(c == 0), stop=(c == EC - 1))
                            if flags["bk"]:
                                nc.vector.tensor_scalar(kTs[:, m, n0:n0 + w], ps[:, :w],
                                                        bk_s[:, m:m + 1], None, ALU.add)
                            else:
                                nc.vector.tensor_copy(kTs[:, m, n0:n0 + w], ps[:, :w])
                # V natural: [128(keyrow), LC, H, 0:64], col 64 = 1.0
                nc.vector.memset(Vs[:, :, :, 64:65], 1.0)
                for r in range(KC if stage_limit >= 2 else 0):
                    for n0 in range(0, E, 512):
                        w = min(512, E - n0)
                        ps = gpsum.tile([128, 512], F32, tag="gp", name=f"psv{r}{n0}")
                        for c in range(EC):
                            nc.tensor.matmul(ps[:, :w], zT[:, c, ts(r, 128)],
                                             wv_s[:, c, n0:n0 + w],
                                             start=(c == 0),
                                             stop=(c == EC - 1 and not flags["bv"]))
                        if flags["bv"]:
                            nc.tensor.matmul(ps[:, :w], ones_row[:], bv_s[:, n0:n0 + w],
                                             start=False, stop=True)
                        h0 = n0 // 64
                        nh = w // 64
                        nc.vector.tensor_copy(
                            Vs[:, r, h0:h0 + nh, 0:64],
                            ps[:, :w].rearrange("p (h d) -> p h d", d=64))

                # ------- attention -------
                with (
                    tc.tile_pool(name="att", bufs=3) as att,
                    tc.tile_pool(name="spsum", bufs=2, space="PSUM") as spsum,
                    tc.tile_pool(name="ypsum", bufs=2, space="PSUM") as ypsum,
                ):
                    from contextlib import nullcontext
                    gvar = None if bake_g is not None else nc.partition_id() % cfg.NPOS
                    for g in range(cfg.NPOS if (stage_limit >= 3 and "noatt" not in ablate) else 0):
                        if bake_g is not None and g != bake_g:
                            continue
                        kc_need = max(kc_of(cfg, j, jmax) for j in pairs[g])
                        if kc_need == 0 and bake_g is None:
                            continue
                        with (nullcontext() if bake_g is not None else tc.If(gvar == g)):
                            emit_kT(kc_need * 128, g)
                            for slot in range(NBPC):
                                j = pairs[g][slot]
                                kc = kc_of(cfg, j, jmax)
                                if kc == 0:
                                    continue
                                qsl = ds(slot * BS, BS)
                                for hp in range(HC):
                                    ps_ys = []
                                    for h01 in (0, 1):
                                        ps_y = ypsum.tile([65, BS], F32, tag="y",
                                                          name=f"y{g}{slot}{hp}{h01}")
                                        ps_ys.append(ps_y)
                                    kdone = 0
                                    while kdone < kc:
                                        gsz = min(4, kc - kdone)
                                        for h01 in (0, 1):
                                            h = 2 * hp + h01
                                            pb = h01 * 64
                                            ps_s = spsum.tile([128, 4, BS], F32, tag="s",
                                                              name=f"s{g}{slot}{hp}{h01}{kdone}")
                                            for i in range(gsz):
                                                ki = kdone + i
                                                nc.tensor.matmul(
                                                    ps_s[:, i, :],
                                                    kTs[pb:pb + 64, hp, ts(ki, 128)],
                                                    qTs[pb:pb + 64, hp, qsl],
                                                    start=True, stop=True)
                                            ex = att.tile([128, 4, BS], BF16, tag="ex",
                                                          name=f"ex{g}{slot}{hp}{h01}{kdone}")
                                            nc.scalar.activation(ex[:, :gsz, :], ps_s[:, :gsz, :], AF.Exp)
                                            for i in range(gsz):
                                                ki = kdone + i
                                                if ki == kc - 2:
                                                    nc.vector.tensor_tensor(ex[:, i, :], ex[:, i, :], maskA[:], ALU.mult)
                                                elif ki == kc - 1:
                                                    nc.vector.tensor_tensor(ex[:, i, :], ex[:, i, :], maskB[:], ALU.mult)
                                            for i in range(gsz):
                                                ki = kdone + i
                                                nc.tensor.matmul(
                                                    ps_ys[h01][:],
                                                    Vs[:, ki, h, :],
                                                    ex[:, i, :],
                                                    start=(ki == 0), stop=(ki == kc - 1))
                                        kdone += gsz
                                    for h01 in (0, 1):
                                        pb = h01 * 64
                                        rr = att.tile([1, BS], F32, tag="rr",
                                                      name=f"rr{g}{slot}{hp}{h01}")
                                        nc.vector.reciprocal(rr[:], ps_ys[h01][64:65, :])
                                        rbp = spsum.tile([128, 4, BS], F32, tag="s",
                                                         name=f"rb{g}{slot}{hp}{h01}")
                                        nc.tensor.matmul(rbp[0:64, 0, :], ones_rf[:], rr[:],
                                                         start=True, stop=True)
                                        rbs = att.tile([64, BS], F32, tag="rbs",
                                                       name=f"rbs{g}{slot}{hp}{h01}")
                                        nc.vector.tensor_copy(rbs[:], rbp[0:64, 0, :])
                                        nc.vector.tensor_tensor(yT[pb:pb + 64, hp, qsl],
                                                                ps_ys[h01][0:64, :],
                                                                rbs[:], ALU.mult)

                # vbar = mean over all L rows of V, per head -> vbarT [128, EC, 1]
                HG = 512 // 65            # heads per vbar psum group
                for h0 in range(0, H if stage_limit >= 4 else 0, HG):
                    nh = min(HG, H - h0)
                    ps = gpsum.tile([1, 512], F32, tag="gp", name=f"vb{h0}")
                    for r in range(KC):
                        nc.tensor.matmul(ps[:, :nh * 65], ones_col[:],
                                         Vs[:, r, h0:h0 + nh, :],
                                         start=(r == 0), stop=(r == KC - 1))
                    nc.vector.tensor_scalar_mul(
                        vrow[:, h0 * 64:(h0 + nh) * 64].rearrange("p (h d) -> p h d", d=64),
                        ps[:, :nh * 65].rearrange("p (h c) -> p h c", c=65)[:, :, 0:64],
                        1.0 / L)
                if KEYS < L and stage_limit >= 4:
                    # tail rows [KEYS, L): vbar += (sum of z rows) @ Wv / L
                    zsum = l2.tile([128, EC, 1], F32, tag="zsum", name="zsum")
                    for c in range(EC):
                        nc.vector.tensor_reduce(zsum[:, c, :], zT[:, c, KEYS:L],
                                                mybir.AxisListType.X, ALU.add)
                    zsumb = l2.tile([128, EC, 1], BF16, tag="zsumb", name="zsumb")
                    nc.vector.tensor_scalar_mul(zsumb[:], zsum[:], 1.0 / L)
                for m in range(EC if stage_limit >= 4 else 0):
                    ps = gpsum.tile([128, 512], F32, tag="gp", name=f"vbt{m}")
                    nc.tensor.matmul(ps[:, 0:1], vrow[:, ts(m, 128)], ones_11[:],
                                     start=True, stop=(KEYS >= L))
                    if KEYS < L:
                        for c in range(EC):
                            nc.tensor.matmul(ps[:, 0:1], wv_s[:, c, ts(m, 128)],
                                             zsumb[:, c, :],
                                             start=False, stop=(c == EC - 1))
                    nc.vector.tensor_copy(vbarT[:, m, :], ps[:, 0:1])
                if KEYS < L and flags["bv"] and stage_limit >= 4:
                    # tail bias: vbar += (L-KEYS)/L * bv  (per hd on partitions)
                    bvt = l2.tile([128, EC, 1], BF16, tag="bvt", name="bvt")
                    nc.sync.dma_start(bvt[:], d_bv.rearrange("o (c p) -> p c o", p=128))
                    nc.vector.tensor_scalar(bvt[:], bvt[:], float(L - KEYS) / L, None,
                                            ALU.mult)
                    nc.vector.tensor_tensor(vbarT[:], vbarT[:], bvt[:], ALU.add)

                # blend: yT = vbar + (yT - vbar) * sel
                vb_b = vbarT[:].to_broadcast([128, EC, R])
                sel_b = selb_s[:, None, :].to_broadcast([128, EC, R])
                nc.vector.tensor_tensor(yT[:], yT[:], vb_b, ALU.subtract)
                nc.vector.tensor_tensor(yT[:], yT[:], sel_b, ALU.mult)
                nc.vector.tensor_tensor(yT[:], yT[:], vb_b, ALU.add)

            # ------- L2c: proj / LN2 / MLP -------
            with tc.tile_pool(name="l2c", bufs=1) as l2c:
                x1T = l2c.tile([128, EC, R], F32)
                x1b = l2c.tile([128, EC, R], BF16)
                z2T = l2c.tile([128, EC, R], BF16)
                hT = l2c.tile([128, FC, R], BF16)
                outT = l2c.tile([128, EC, R], F32)

                for m in range(EC if stage_limit >= 5 else 0):
                    ps = gpsum.tile([128, 512], F32, tag="gp", name=f"psp{m}")
                    for c in range(HC):
                        nc.tensor.matmul(ps[:, :R], wp_s[:, c, ts(m, 128)], yT[:, c, :],
                                         start=(c == 0), stop=(c == HC - 1))
                    nc.vector.tensor_tensor(x1T[:, m, :], ps[:, :R], xo_s[:, m, :], ALU.add)
                    if flags["bproj"]:
                        nc.vector.tensor_scalar(x1T[:, m, :], x1T[:, m, :],
                                                bp_s[:, m:m + 1], None, ALU.add)
                    nc.vector.tensor_copy(x1b[:, m, :], x1T[:, m, :])

                if stage_limit >= 5:
                    with tc.tile_pool(name="l3c", bufs=1) as l3c, \
                         tc.tile_pool(name=f"bp2{ri}", bufs=2, space="PSUM") as bp2:
                        ln_transposed(l3c, x1b, R, (2, 3), z2T, "2", bp2)

                with tc.tile_pool(name="wstream", bufs=2) as wstream:
                    for m in range(FC if (stage_limit >= 6 and "nomlp" not in ablate) else 0):
                        wfc_m = wstream.tile([128, EC, 128], BF16, tag="wfc", name=f"wfc{m}")
                        nc.sync.dma_start(wfc_m[:], d_wfc[m].rearrange("p (c n) -> p c n", c=EC))
                        ps = gpsum.tile([128, 512], F32, tag="gp", name=f"psh{m}")
                        for c in range(EC):
                            nc.tensor.matmul(ps[:, :R], wfc_m[:, c, :], z2T[:, c, :],
                                             start=(c == 0), stop=(c == EC - 1))
                        sg = wstream.tile([128, R], BF16, tag="sg", name=f"sg{m}")
                        if flags["bfc"]:
                            nc.scalar.activation(sg[:], ps[:, :R], AF.Sigmoid,
                                                 bias=bfc_s[:, m:m + 1])
                            t2 = wstream.tile([128, R], F32, tag="t2", name=f"t2{m}")
                            nc.vector.tensor_scalar(t2[:], ps[:, :R],
                                                    bfc_s[:, m:m + 1], None, ALU.add)
                            nc.vector.tensor_tensor(hT[:, m, :], t2[:], sg[:], ALU.mult)
                        else:
                            nc.scalar.activation(sg[:], ps[:, :R], AF.Sigmoid)
                            nc.vector.tensor_tensor(hT[:, m, :], ps[:, :R], sg[:], ALU.mult)
                    for m in range(EC if stage_limit >= 6 else 0):
                        wout_m = wstream.tile([128, FC, 128], BF16, tag="wout", name=f"wout{m}")
                        nc.sync.dma_start(wout_m[:], d_wout[m].rearrange("p (k n) -> p k n", k=FC))
                        ps = gpsum.tile([128, 512], F32, tag="gp", name=f"pso{m}")
                        for k in range(FC):
                            nc.tensor.matmul(ps[:, :R], wout_m[:, k, :], hT[:, k, :],
                                             start=(k == 0), stop=(k == FC - 1))
                        nc.vector.tensor_tensor(outT[:, m, :], ps[:, :R], x1T[:, m, :], ALU.add)
                        if flags["bout"]:
                            nc.vector.tensor_scalar(outT[:, m, :], outT[:, m, :],
                                                    bout_s[:, m:m + 1], None, ALU.add)

                nc.sync.dma_start(d_out.rearrange("p (c n) -> p c n", c=EC), outT[:])

        if loop_n > 1:
            with tc.For_i(0, loop_n, 1):
                emit_body(0)
        else:
            for _ri in range(repeat):
                emit_body(_ri)

    nc.finalize()
    return nc


# ----------------------------------------------------------------------------
# host side: input prep / output assembly
# ----------------------------------------------------------------------------

def prepare_in_maps(cfg: Cfg, pairs, jmax, flags, inputs):
    """Build per-core input maps. Returns (in_maps, percore_blocks)."""
    x = np.asarray(inputs["x"], np.float32)
    w_qkv = np.asarray(inputs["w_qkv"], np.float32)
    b_qkv = np.asarray(inputs["b_qkv"], np.float32)
    w_proj = np.asarray(inputs["w_proj"], np.float32)
    b_proj = np.asarray(inputs["b_proj"], np.float32)
    w_fc = np.asarray(inputs["w_fc"], np.float32)
    b_fc = np.asarray(inputs["b_fc"], np.float32)
    w_out = np.asarray(inputs["w_out"], np.float32)
    b_out = np.asarray(inputs["b_out"], np.float32)
    ln1_s = np.asarray(inputs["ln1_scale"], np.float32)
    ln1_b = np.asarray(inputs["ln1_bias"], np.float32)
    ln2_s = np.asarray(inputs["ln2_scale"], np.float32)
    ln2_b = np.asarray(inputs["ln2_bias"], np.float32)
    mask_len = np.asarray(inputs["mask_len"]).astype(np.int64)

    E, L, H, D, BS = cfg.E, cfg.L, cfg.H, cfg.D, cfg.BS
    EC, FC = cfg.EC, cfg.FC

    # split qkv columns: col = h*3D + {0..D-1:q, D..2D-1:k, 2D..3D-1:v}
    wsplit = w_qkv.reshape(E, H, 3 * D)
    wq = np.ascontiguousarray(wsplit[:, :, 0:D].reshape(E, E))
    wk = np.ascontiguousarray(wsplit[:, :, D:2 * D].reshape(E, E))
    wv = np.ascontiguousarray(wsplit[:, :, 2 * D:3 * D].reshape(E, E))
    bsplit = b_qkv.reshape(H, 3 * D)
    bq = np.ascontiguousarray(bsplit[:, 0:D].reshape(E))
    bk = np.ascontiguousarray(bsplit[:, D:2 * D].reshape(E))
    bv = np.ascontiguousarray(bsplit[:, 2 * D:3 * D].reshape(E))

    def chunked_w(w):  # [E, N] -> partition-major [128, EC*N] bf16
        n = w.shape[1]
        return np.ascontiguousarray(
            w.reshape(EC, 128, n).transpose(1, 0, 2).reshape(128, EC * n)).astype(BF16NP)

    def col_f32(v):    # [E or FF] -> [128, C]
        return np.ascontiguousarray(v.reshape(-1, 128).T).astype(np.float32)

    wq_c, wk_c, wv_c, wp_c = (chunked_w(w) for w in (wq, wk, wv, w_proj))
    wfc_c = np.ascontiguousarray(
        w_fc.reshape(EC, 128, FC, 128).transpose(2, 1, 0, 3).reshape(FC, 128, EC * 128)
    ).astype(BF16NP)
    wout_c = np.ascontiguousarray(
        w_out.reshape(FC, 128, EC, 128).transpose(2, 1, 0, 3).reshape(EC, 128, FC * 128)
    ).astype(BF16NP)
    lnp = np.ascontiguousarray(np.stack(
        [col_f32(ln1_s), col_f32(ln1_b), col_f32(ln2_s), col_f32(ln2_b)]
    ).transpose(1, 0, 2))

    ki = np.arange(128)[:, None]
    qi = np.arange(BS)[None, :]
    masks = np.stack([(qi >= ki), (qi >= ki + 128)]).astype(BF16NP)

    shared = dict(
        wq=wq_c, wk=wk_c, wv=wv_c, wproj=wp_c, wfc=wfc_c, wout=wout_c,
        bq=col_f32(bq), bk=col_f32(bk), bv=bv.reshape(1, E).astype(BF16NP),
        bproj=col_f32(b_proj), bfc=col_f32(b_fc), bout=col_f32(b_out),
        lnp=lnp, diagmasks=masks,
    )

    in_maps = []
    percore_blocks = []
    for c in range(cfg.n_cores):
        b = c // cfg.NPOS
        g = c % cfg.NPOS
        blocks = pairs[g]
        percore_blocks.append((b, blocks))
        xT = x[b].T  # [E, L]
        own_cols = np.concatenate(
            [np.arange(j * BS, (j + 1) * BS) for j in blocks])
        sel = (own_cols < mask_len[b]).astype(BF16NP)
        selb = np.broadcast_to(sel[None, :], (128, cfg.R))
        E_, L_ = xT.shape
        EC_ = E_ // 128
        m = dict(shared)
        m["xT_full"] = np.ascontiguousarray(
            xT.reshape(EC_, 128, L_).transpose(1, 0, 2).reshape(128, EC_ * L_)).astype(BF16NP)
        xo = xT[:, own_cols]
        m["xT_own"] = np.ascontiguousarray(
            xo.reshape(EC_, 128, -1).transpose(1, 0, 2).reshape(128, -1)).astype(np.float32)
        m["selb"] = np.ascontiguousarray(selb)
        in_maps.append(m)
    return in_maps, percore_blocks


def assemble_output(cfg: Cfg, results, percore_blocks):
    out = np.zeros((cfg.B, cfg.L, cfg.E), np.float32)
    for c, res in enumerate(results):
        b, blocks = percore_blocks[c]
        oT = res["outT"].reshape(128, cfg.EC, cfg.R).transpose(1, 0, 2).reshape(cfg.E, cfg.R)
        for s, j in enumerate(blocks):
            out[b, j * cfg.BS:(j + 1) * cfg.BS, :] = oT[:, s * cfg.BS:(s + 1) * cfg.BS].T
    return out


def make_flags(inputs):
    def nz(name):
        return bool(np.any(np.asarray(inputs[name]) != 0))
    return dict(
        bq=nz("b_qkv"), bk=nz("b_qkv"), bv=nz("b_qkv"),
        bproj=nz("b_proj"), bfc=nz("b_fc"), bout=nz("b_out"),
        ln1aff=bool(np.any(np.asarray(inputs["ln1_scale"]) != 1)
                    or np.any(np.asarray(inputs["ln1_bias"]) != 0)),
        ln2aff=bool(np.any(np.asarray(inputs["ln2_scale"]) != 1)
                    or np.any(np.asarray(inputs["ln2_bias"]) != 0)),
    )


_cached = {}


def kernel(**inputs) -> np.ndarray:
    cfg = Cfg()
    mask_len = np.asarray(inputs["mask_len"]).astype(np.int64)
    pairs, jmax = plan_blocks(cfg, mask_len)
    flags = make_flags(inputs)
    key = (tuple(map(tuple, pairs)), jmax, tuple(sorted(flags.items())))
    if key not in _cached:
        _cached[key] = build_program(cfg, pairs, jmax, flags)
    nc = _cached[key]
    in_maps, percore_blocks = prepare_in_maps(cfg, pairs, jmax, flags, inputs)
    r = run_bass_kernel_spmd(nc, in_maps, core_ids=list(range(cfg.n_cores)))
    return assemble_output(cfg, r.results, percore_blocks)


if __name__ == "__main__":
    pass



# revision 15
# speedup vs baseline: 150.8303x; 150.8303x over previous
"""Trainium2 Bass kernel for nn_AttentionBlock (pre-LN causal attention + SiLU MLP).

8-core SPMD strategy (data-parallel over batch x sequence-parallel over rows):
  - core c handles sample b = c // NPOS, position g = c % NPOS
  - the L rows of a sample are split into NBLK blocks of BS rows; each core owns
    NBPC blocks, paired to balance causal-attention cost (host-chosen pairing)
  - every core computes LN1 + K^T/V for the full sample (replicated), q/proj/MLP
    only for its own rows.  Per-core differences in causal trip counts are
    handled with tc.If branches on partition_id; per-core data differences
    (mask_len row selection) are handled via input data (sel blend / vbar).

All matmul layouts are "transposed" (feature dim on partitions) so no on-device
transposes are needed anywhere; the host feeds x pre-transposed and re-assembles
the transposed output.
"""
import math
from contextlib import ExitStack
from dataclasses import dataclass

import ml_dtypes
import numpy as np

import concourse.bass as bass
import concourse.mybir as mybir
import concourse.tile as tile
from concourse import bacc
from concourse.bass import ds, ts
from concourse.bass_utils import run_bass_kernel_spmd

F32 = mybir.dt.float32
BF16 = mybir.dt.bfloat16
AF = mybir.ActivationFunctionType
ALU = mybir.AluOpType
BF16NP = ml_dtypes.bfloat16


@dataclass
class Cfg:
    B: int = 2
    L: int = 2048
    E: int = 768
    H: int = 12
    D: int = 64
    FF: int = 3072
    BS: int = 256          # query block rows
    n_cores: int = 8
    eps: float = 1e-6

    @property
    def NPOS(self):
        return self.n_cores // self.B

    @property
    def NBLK(self):
        return self.L // self.BS

    @property
    def NBPC(self):
        return self.NBLK // self.NPOS   # blocks per core

    @property
    def R(self):
        return self.NBPC * self.BS      # own rows per core

    @property
    def EC(self):
        return self.E // 128

    @property
    def FC(self):
        return self.FF // 128

    @property
    def LC(self):
        return self.L // 128

    @property
    def HC(self):
        return self.H // 2              # head-pair chunks (= EC since E = H*D, D=64)


def plan_blocks(cfg: Cfg, mask_lens):
    """Choose jmax (number of attention-active blocks) and block pairing."""
    mmax = int(max(int(m) for m in mask_lens))
    mmax = max(1, min(cfg.L, mmax))
    jmax = (mmax + cfg.BS - 1) // cfg.BS          # blocks [0, jmax) need causal attn
    def cost(j):
        return (j + 1) if j < jmax else 0
    order = sorted(range(cfg.NBLK), key=lambda j: -cost(j))
    pairs = []
    for g in range(cfg.NPOS):
        blocks = []
        for s in range(cfg.NBPC):
            # snake over sorted order: pair heavy with light
            idx = g if s % 2 == 0 else (cfg.NBLK - 1 - g)
            blocks.append(order[idx])
        pairs.append(tuple(blocks))
    return pairs, jmax


def kc_of(cfg: Cfg, j, jmax):
    """number of 128-wide key chunks block j attends to (0 if mask-free)."""
    if j >= jmax:
        return 0
    return (j + 1) * cfg.BS // 128


def slot_kcs(cfg: Cfg, pairs, jmax):
    """Uniform (branch-free) per-slot key-chunk trip counts: the max over
    cores. Cores whose block needs fewer chunks mask the excess to zero via
    the per-core attnmask input; fully-masked blocks compute garbage that the
    sel blend discards (their denominators stay positive: plain causal
    masks are supplied for every assigned block, active or not)."""
    return [max(kc_of(cfg, pairs[g][s], jmax) for g in range(len(pairs)))
            for s in range(cfg.NBPC)]


# ----------------------------------------------------------------------------
# program builder
# ----------------------------------------------------------------------------

def build_program(cfg: Cfg, pairs, jmax, flags, bake_g=None, stage_limit=99, repeat=1,
                  loop_n=1, ablate=()):
    """flags: dict with bools: bq, bk, bv, bproj, bfc, bout, ln1aff, ln2aff

    bake_g: if set, emit only that variant's attention without tc.If (for
    timing estimation with TimelineSim)."""
    E, L, H, D2, FF, BS, R = cfg.E, cfg.L, cfg.H, cfg.D, cfg.FF, cfg.BS, cfg.R
    EC, FC, LC, HC, NBPC = cfg.EC, cfg.FC, cfg.LC, cfg.HC, cfg.NBPC
    KEYS = jmax * BS
    KC = KEYS // 128
    KS = slot_kcs(cfg, pairs, jmax)      # uniform per-slot trip counts
    NK = sum(KS)
    qscale = 1.0 / math.sqrt(cfg.D)

    nc = bacc.Bacc(num_devices=cfg.n_cores)

    # ---- dram I/O ----
    d_xTf = nc.dram_tensor("xT_full", [128, EC * L], BF16, kind="ExternalInput")
    d_xTo = nc.dram_tensor("xT_own", [128, EC * R], F32, kind="ExternalInput")
    d_wq = nc.dram_tensor("wq", [128, EC * E], BF16, kind="ExternalInput")
    d_wk = nc.dram_tensor("wk", [128, EC * E], BF16, kind="ExternalInput")
    d_wv = nc.dram_tensor("wv", [128, EC * E], BF16, kind="ExternalInput")
    d_wp = nc.dram_tensor("wproj", [128, EC * E], BF16, kind="ExternalInput")
    d_wfc = nc.dram_tensor("wfc", [FC, 128, EC * 128], BF16, kind="ExternalInput")
    d_wout = nc.dram_tensor("wout", [EC, 128, FC * 128], BF16, kind="ExternalInput")
    d_bq = nc.dram_tensor("bq", [128, EC], F32, kind="ExternalInput")
    d_bk = nc.dram_tensor("bk", [128, EC], F32, kind="ExternalInput")
    d_bv = nc.dram_tensor("bv", [1, E], BF16, kind="ExternalInput")
    d_bp = nc.dram_tensor("bproj", [128, EC], F32, kind="ExternalInput")
    d_bfc = nc.dram_tensor("bfc", [128, FC], F32, kind="ExternalInput")
    d_bout = nc.dram_tensor("bout", [128, EC], F32, kind="ExternalInput")
    d_ln = nc.dram_tensor("lnp", [128, 4, EC], F32, kind="ExternalInput")  # g1,b1,g2,b2
    d_selb = nc.dram_tensor("selb", [128, R], BF16, kind="ExternalInput")
    d_amask = nc.dram_tensor("attnmask", [NK, 128, BS], BF16, kind="ExternalInput")
    d_out = nc.dram_tensor("outT", [128, EC * R], F32, kind="ExternalOutput")

    with tile.TileContext(nc) as tc, ExitStack() as st:
        # ------- L0: persistent pools -------
        cpool = st.enter_context(tc.tile_pool(name="consts", bufs=1))

        wq_s = cpool.tile([128, EC, E], BF16)
        wk_s = cpool.tile([128, EC, E], BF16)
        wv_s = cpool.tile([128, EC, E], BF16)
        wp_s = cpool.tile([128, EC, E], BF16)
        xo_s = cpool.tile([128, EC, R], F32)
        bq_s = cpool.tile([128, EC], F32)
        bk_s = cpool.tile([128, EC], F32)
        bv_s = cpool.tile([1, E], BF16)
        bp_s = cpool.tile([128, EC], F32)
        bfc_s = cpool.tile([128, FC], F32)
        bout_s = cpool.tile([128, EC], F32)
        ln_s = cpool.tile([128, 4, EC], F32)
        selb_s = cpool.tile([128, R], BF16)
        amask_s = cpool.tile([128, NK, BS], BF16)
        ones_col = cpool.tile([128, 1], BF16)
        ones_row = cpool.tile([1, 128], BF16)
        ones_rf = cpool.tile([1, 64], F32)
        ones_11 = cpool.tile([1, 1], BF16)
        eps_11 = cpool.tile([1, 1], F32)
        yT = cpool.tile([128, HC, R], BF16)
        vbarT = cpool.tile([128, EC, 1], F32)
        vrow = cpool.tile([1, E], BF16)

      # loop body emitted `repeat` times (timing calibration); closures below
      # reference gpsum, set per-iteration.
      # NOTE: 6-space indent keeps the original body indentation valid.

        def emit_body(ri):
          def pbcast(out_ap, in_ap, ch):
              if "nobc" in ablate:
                  nc.vector.memset(out_ap, 1.0)
              else:
                  nc.gpsimd.partition_broadcast(out_ap, in_ap, channels=ch)
          with tc.tile_pool(name=f"gpsum{ri}", bufs=2, space="PSUM") as gpsum:
            nc.sync.dma_start(wq_s[:], d_wq.rearrange("p (c n) -> p c n", c=EC))
            nc.sync.dma_start(wk_s[:], d_wk.rearrange("p (c n) -> p c n", c=EC))
            nc.sync.dma_start(wv_s[:], d_wv.rearrange("p (c n) -> p c n", c=EC))
            nc.sync.dma_start(wp_s[:], d_wp.rearrange("p (c n) -> p c n", c=EC))
            nc.sync.dma_start(xo_s[:], d_xTo.rearrange("p (c n) -> p c n", c=EC))
            nc.sync.dma_start(bq_s[:], d_bq[:])
            nc.sync.dma_start(bk_s[:], d_bk[:])
            nc.sync.dma_start(bv_s[:], d_bv[:])
            nc.sync.dma_start(bp_s[:], d_bp[:])
            nc.sync.dma_start(bfc_s[:], d_bfc[:])
            nc.sync.dma_start(bout_s[:], d_bout[:])
            nc.sync.dma_start(ln_s[:], d_ln[:])
            nc.sync.dma_start(selb_s[:], d_selb[:])
            nc.sync.dma_start(amask_s[:], d_amask.rearrange("k p q -> p k q"))
            nc.vector.memset(ones_col[:], 1.0)
            nc.vector.memset(ones_row[:], 1.0)
            nc.vector.memset(ones_rf[:], 1.0)
            nc.vector.memset(ones_11[:], 1.0)
            nc.vector.memset(eps_11[:], cfg.eps)
            nc.vector.memset(yT[:], 0.0)
            # ============================================================
            # helper: layernorm in transposed layout
            # x_bf: sbuf [128, EC, N] bf16 ; writes zT [128, EC, N] bf16
            # ============================================================
            def ln_transposed(pool, x_bf, N, gb_idx, zT_out, tag, bpool):
                """zT_out may alias x_bf (in-place LN apply)."""
                gi, bi = gb_idx
                affine = flags["ln1aff"] if gb_idx == (0, 1) else flags["ln2aff"]
                for cg0 in range(0, N, 512):
                    w = min(512, N - cg0)
                    ps_su = gpsum.tile([1, 512], F32, tag="gp", name=f"pssu{tag}{cg0}")
                    ps_sq = gpsum.tile([1, 512], F32, tag="gp", name=f"pssq{tag}{cg0}")
                    for c in range(EC):
                        nc.tensor.matmul(ps_su[:, :w], ones_col[:], x_bf[:, c, cg0:cg0 + w],
                                         start=(c == 0), stop=(c == EC - 1))
                    for c in range(EC):
                        sq = pool.tile([128, 512], BF16, tag="lnsq", name=f"sq{tag}{cg0}{c}")
                        nc.vector.tensor_tensor(sq[:, :w], x_bf[:, c, cg0:cg0 + w],
                                                x_bf[:, c, cg0:cg0 + w], ALU.mult)
                        nc.tensor.matmul(ps_sq[:, :w], ones_col[:], sq[:, :w],
                                         start=(c == 0), stop=(c == EC - 1))
                    # mu = sum/E ; var = sumsq/E - mu^2 ; a = 1/sqrt(var+eps) ; b = -mu*a
                    mu = pool.tile([1, 512], F32, tag="lnmu", name=f"mu{tag}{cg0}")
                    nc.vector.tensor_scalar_mul(mu[:, :w], ps_su[:, :w], 1.0 / E)
                    va = pool.tile([1, 512], F32, tag="lnva", name=f"va{tag}{cg0}")
                    nc.vector.tensor_scalar_mul(va[:, :w], ps_sq[:, :w], 1.0 / E)
                    t1 = pool.tile([1, 512], F32, tag="lnt1", name=f"t1{tag}{cg0}")
                    nc.vector.tensor_tensor(t1[:, :w], mu[:, :w], mu[:, :w], ALU.mult)
                    nc.vector.tensor_sub(va[:, :w], va[:, :w], t1[:, :w])
                    nc.scalar.activation(t1[:, :w], va[:, :w], AF.Sqrt, bias=eps_11[:])
                    nc.vector.reciprocal(va[:, :w], t1[:, :w])     # va = rstd = a
                    arow = pool.tile([1, 512], BF16, tag="lnar", name=f"ar{tag}{cg0}")
                    nc.vector.tensor_copy(arow[:, :w], va[:, :w])
                    nc.vector.tensor_tensor(t1[:, :w], mu[:, :w], va[:, :w], ALU.mult)
                    brow = pool.tile([1, 512], BF16, tag="lnbr", name=f"br{tag}{cg0}")
                    nc.vector.tensor_scalar_mul(brow[:, :w], t1[:, :w], -1.0)
                    ab = bpool.tile([128, 2, 512], F32, tag="lnab", name=f"ab{tag}{cg0}")
                    nc.tensor.matmul(ab[:, 0, :w], ones_row[:], arow[:, :w],
                                     start=True, stop=True)
                    nc.tensor.matmul(ab[:, 1, :w], ones_row[:], brow[:, :w],
                                     start=True, stop=True)
                    for c in range(EC):
                        nc.vector.tensor_tensor(zT_out[:, c, cg0:cg0 + w],
                                                x_bf[:, c, cg0:cg0 + w], ab[:, 0, :w], ALU.mult)
                        nc.vector.tensor_tensor(zT_out[:, c, cg0:cg0 + w],
                                                zT_out[:, c, cg0:cg0 + w], ab[:, 1, :w], ALU.add)
                        if affine:
                            nc.vector.tensor_scalar(zT_out[:, c, cg0:cg0 + w],
                                                    zT_out[:, c, cg0:cg0 + w],
                                                    ln_s[:, gi, c:c + 1], ln_s[:, bi, c:c + 1],
                                                    ALU.mult, ALU.add)

            # ------- L2: sample-wide tensors (die after attention) -------
            with tc.tile_pool(name="l2", bufs=1) as l2:
                # zT / zqT are computed in place over the loaded x tiles
                zT = l2.tile([128, EC, L], BF16, tag="zT", name="zT")
                nc.sync.dma_start(zT[:], d_xTf.rearrange("p (c n) -> p c n", c=EC))
                zqT = l2.tile([128, EC, R], BF16, tag="zqT", name="zqT")
                nc.vector.tensor_copy(zqT[:], xo_s[:])
                qTs = l2.tile([128, HC, R], BF16, tag="qTs", name="qTs")
                kTs = l2.tile([128, HC, KEYS], BF16, tag="kTs", name="kTs")
                Vs = l2.tile([128, KC, H, 65], BF16, tag="Vs", name="Vs")

                # ------- L3: LN1 scratch (dies after zT/zqT written) -------
                if stage_limit >= 1 and "noln1" not in ablate:
                    with tc.tile_pool(name="l3", bufs=2) as l3, \
                         tc.tile_pool(name=f"bp1{ri}", bufs=2, space="PSUM") as bp1:
                        ln_transposed(l3, zT, L, (0, 1), zT, "f", bp1)
                        ln_transposed(l3, zqT, R, (0, 1), zqT, "o", bp1)

                # ------- QKV -------
                if "noqkv" in ablate:
                    nc.vector.memset(qTs[:], 0.001)
                    nc.vector.memset(kTs[:], 0.001)
                # q^T (own rows): [128(hd), HC, R]
                for m in range(EC if (stage_limit >= 2 and "noqkv" not in ablate) else 0):
                    ps = gpsum.tile([128, 512], F32, tag="gp", name=f"psq{m}")
                    for c in range(EC):
                        nc.tensor.matmul(ps[:, :R], wq_s[:, c, ts(m, 128)], zqT[:, c, :],
                                         start=(c == 0), stop=(c == EC - 1))
                    if flags["bq"]:
                        nc.vector.tensor_scalar(qTs[:, m, :], ps[:, :R], bq_s[:, m:m + 1],
                                                qscale, ALU.add, ALU.mult)
                    else:
                        nc.vector.tensor_scalar_mul(qTs[:, m, :], ps[:, :R], qscale)
                # k^T (keys 0..keys_g): [128(hd), HC, KEYS] — emitted per variant
                # so each core only computes the key range its blocks reach
                def emit_kT(keys_g, gtag):
                    if stage_limit < 2 or "noqkv" in ablate:
                        return
                    for m in range(EC):
                        for n0 in range(0, keys_g, 512):
                            w = min(512, keys_g - n0)
                            ps = gpsum.tile([128, 512], F32, tag="gp",
                                            name=f"psk{gtag}{m}{n0}")
                            for c in range(EC):
                                nc.tensor.matmul(ps[:, :w], wk_s[:, c, ts(m, 128)],
                                                 zT[:, c, n0:n0 + w],
                                                 start=(c == 0), stop=(c == EC - 1))
                            if flags["bk"]:
                                nc.vector.tensor_scalar(kTs[:, m, n0:n0 + w], ps[:, :w],
                                                        bk_s[:, m:m + 1], None, ALU.add)
                            else:
                                nc.vector.tensor_copy(kTs[:, m, n0:n0 + w], ps[:, :w])
                # V natural: [128(keyrow), LC, H, 0:64], col 64 = 1.0
                nc.vector.memset(Vs[:, :, :, 64:65], 1.0)
                if "noqkv" in ablate:
                    nc.vector.memset(Vs[:, :, :, 0:64], 0.001)
                for r in range(KC if (stage_limit >= 2 and "noqkv" not in ablate) else 0):
                    for n0 in range(0, E, 512):
                        w = min(512, E - n0)
                        ps = gpsum.tile([128, 512], F32, tag="gp", name=f"psv{r}{n0}")
                        for c in range(EC):
                            nc.tensor.matmul(ps[:, :w], zT[:, c, ts(r, 128)],
                                             wv_s[:, c, n0:n0 + w],
                                             start=(c == 0),
                                             stop=(c == EC - 1 and not flags["bv"]))
                        if flags["bv"]:
                            nc.tensor.matmul(ps[:, :w], ones_row[:], bv_s[:, n0:n0 + w],
                                             start=False, stop=True)
                        h0 = n0 // 64
                        nh = w // 64
                        nc.vector.tensor_copy(
                            Vs[:, r, h0:h0 + nh, 0:64],
                            ps[:, :w].rearrange("p (h d) -> p h d", d=64))

                # ------- attention (branch-free: identical instructions on
                # every core; per-core causal masking comes in via d_amask) ---
                with (
                    tc.tile_pool(name="att", bufs=3) as att,
                    tc.tile_pool(name="spsum", bufs=2, space="PSUM") as spsum,
                    tc.tile_pool(name="ypsum", bufs=2, space="PSUM") as ypsum,
                ):
                    if stage_limit >= 3 and "noatt" not in ablate:
                        emit_kT(max(KS) * 128, "u")
                        for slot in range(NBPC):
                            kc = KS[slot]
                            if kc == 0:
                                continue
                            base = sum(KS[:slot])
                            qsl = ds(slot * BS, BS)
                            for hp in range(HC):
                                ps_ys = []
                                for h01 in (0, 1):
                                    ps_y = ypsum.tile([65, BS], F32, tag="y",
                                                      name=f"y{slot}{hp}{h01}")
                                    ps_ys.append(ps_y)
                                kdone = 0
                                while kdone < kc:
                                    gsz = min(4, kc - kdone)
                                    for h01 in (0, 1):
                                        h = 2 * hp + h01
                                        pb = h01 * 64
                                        ps_s = spsum.tile([128, 4, BS], F32, tag="s",
                                                          name=f"s{slot}{hp}{h01}{kdone}")
                                        for i in range(gsz):
                                            ki = kdone + i
                                            nc.tensor.matmul(
                                                ps_s[:, i, :],
                                                kTs[pb:pb + 64, hp, ts(ki, 128)],
                                                qTs[pb:pb + 64, hp, qsl],
                                                start=True, stop=True)
                                        ex = att.tile([128, 4, BS], BF16, tag="ex",
                                                      name=f"ex{slot}{hp}{h01}{kdone}")
                                        nc.scalar.activation(ex[:, :gsz, :], ps_s[:, :gsz, :], AF.Exp)
                                        nc.vector.tensor_tensor(
                                            ex[:, :gsz, :], ex[:, :gsz, :],
                                            amask_s[:, base + kdone:base + kdone + gsz, :],
                                            ALU.mult)
                                        for i in range(gsz):
                                            ki = kdone + i
                                            nc.tensor.matmul(
                                                ps_ys[h01][:],
                                                Vs[:, ki, h, :],
                                                ex[:, i, :],
                                                start=(ki == 0), stop=(ki == kc - 1))
                                    kdone += gsz
                                for h01 in (0, 1):
                                    pb = h01 * 64
                                    rr = att.tile([1, BS], F32, tag="rr",
                                                  name=f"rr{slot}{hp}{h01}")
                                    nc.vector.reciprocal(rr[:], ps_ys[h01][64:65, :])
                                    rbp = spsum.tile([128, 4, BS], F32, tag="s",
                                                     name=f"rb{slot}{hp}{h01}")
                                    nc.tensor.matmul(rbp[0:64, 0, :], ones_rf[:], rr[:],
                                                     start=True, stop=True)
                                    rbs = att.tile([64, BS], F32, tag="rbs",
                                                   name=f"rbs{slot}{hp}{h01}")
                                    nc.vector.tensor_copy(rbs[:], rbp[0:64, 0, :])
                                    nc.vector.tensor_tensor(yT[pb:pb + 64, hp, qsl],
                                                            ps_ys[h01][0:64, :],
                                                            rbs[:], ALU.mult)

                # vbar = mean over all L rows of V, per head -> vbarT [128, EC, 1]
                if "novbar" in ablate:
                    nc.vector.memset(vbarT[:], 0.001)
                HG = 512 // 65            # heads per vbar psum group
                for h0 in range(0, H if (stage_limit >= 4 and "novbar" not in ablate) else 0, HG):
                    nh = min(HG, H - h0)
                    ps = gpsum.tile([1, 512], F32, tag="gp", name=f"vb{h0}")
                    for r in range(KC):
                        nc.tensor.matmul(ps[:, :nh * 65], ones_col[:],
                                         Vs[:, r, h0:h0 + nh, :],
                                         start=(r == 0), stop=(r == KC - 1))
                    nc.vector.tensor_scalar_mul(
                        vrow[:, h0 * 64:(h0 + nh) * 64].rearrange("p (h d) -> p h d", d=64),
                        ps[:, :nh * 65].rearrange("p (h c) -> p h c", c=65)[:, :, 0:64],
                        1.0 / L)
                if KEYS < L and stage_limit >= 4 and 'novbar' not in ablate:
                    # tail rows [KEYS, L): vbar += (sum of z rows) @ Wv / L
                    zsum = l2.tile([128, EC, 1], F32, tag="zsum", name="zsum")
                    for c in range(EC):
                        nc.vector.tensor_reduce(zsum[:, c, :], zT[:, c, KEYS:L],
                                                mybir.AxisListType.X, ALU.add)
                    zsumb = l2.tile([128, EC, 1], BF16, tag="zsumb", name="zsumb")
                    nc.vector.tensor_scalar_mul(zsumb[:], zsum[:], 1.0 / L)
                for m in range(EC if (stage_limit >= 4 and 'novbar' not in ablate) else 0):
                    ps = gpsum.tile([128, 512], F32, tag="gp", name=f"vbt{m}")
                    nc.tensor.matmul(ps[:, 0:1], vrow[:, ts(m, 128)], ones_11[:],
                                     start=True, stop=(KEYS >= L))
                    if KEYS < L:
                        for c in range(EC):
                            nc.tensor.matmul(ps[:, 0:1], wv_s[:, c, ts(m, 128)],
                                             zsumb[:, c, :],
                                             start=False, stop=(c == EC - 1))
                    nc.vector.tensor_copy(vbarT[:, m, :], ps[:, 0:1])
                if KEYS < L and flags["bv"] and stage_limit >= 4 and 'novbar' not in ablate:
                    # tail bias: vbar += (L-KEYS)/L * bv  (per hd on partitions)
                    bvt = l2.tile([128, EC, 1], BF16, tag="bvt", name="bvt")
                    nc.sync.dma_start(bvt[:], d_bv.rearrange("o (c p) -> p c o", p=128))
                    nc.vector.tensor_scalar(bvt[:], bvt[:], float(L - KEYS) / L, None,
                                            ALU.mult)
                    nc.vector.tensor_tensor(vbarT[:], vbarT[:], bvt[:], ALU.add)

                # blend: yT = vbar + (yT - vbar) * sel
                vb_b = vbarT[:].to_broadcast([128, EC, R])
                sel_b = selb_s[:, None, :].to_broadcast([128, EC, R])
                nc.vector.tensor_tensor(yT[:], yT[:], vb_b, ALU.subtract)
                nc.vector.tensor_tensor(yT[:], yT[:], sel_b, ALU.mult)
                nc.vector.tensor_tensor(yT[:], yT[:], vb_b, ALU.add)

            # ------- L2c: proj / LN2 / MLP -------
            with tc.tile_pool(name="l2c", bufs=1) as l2c:
                x1T = l2c.tile([128, EC, R], F32)
                x1b = l2c.tile([128, EC, R], BF16)
                z2T = l2c.tile([128, EC, R], BF16)
                hT = l2c.tile([128, FC, R], BF16)
                outT = l2c.tile([128, EC, R], F32)

                if "noproj" in ablate:
                    nc.vector.memset(x1T[:], 0.001)
                    nc.vector.memset(x1b[:], 0.001)
                for m in range(EC if (stage_limit >= 5 and "noproj" not in ablate) else 0):
                    ps = gpsum.tile([128, 512], F32, tag="gp", name=f"psp{m}")
                    for c in range(HC):
                        nc.tensor.matmul(ps[:, :R], wp_s[:, c, ts(m, 128)], yT[:, c, :],
                                         start=(c == 0), stop=(c == HC - 1))
                    nc.vector.tensor_tensor(x1T[:, m, :], ps[:, :R], xo_s[:, m, :], ALU.add)
                    if flags["bproj"]:
                        nc.vector.tensor_scalar(x1T[:, m, :], x1T[:, m, :],
                                                bp_s[:, m:m + 1], None, ALU.add)
                    nc.vector.tensor_copy(x1b[:, m, :], x1T[:, m, :])

                if "noln2" in ablate:
                    nc.vector.memset(z2T[:], 0.001)
                if stage_limit >= 5 and "noln2" not in ablate:
                    with tc.tile_pool(name="l3c", bufs=1) as l3c, \
                         tc.tile_pool(name=f"bp2{ri}", bufs=2, space="PSUM") as bp2:
                        ln_transposed(l3c, x1b, R, (2, 3), z2T, "2", bp2)

                with tc.tile_pool(name="wstream", bufs=2) as wstream:
                    if "nomlp" in ablate:
                        nc.vector.memset(hT[:], 0.001)
                    for m in range(FC if (stage_limit >= 6 and "nomlp" not in ablate) else 0):
                        wfc_m = wstream.tile([128, EC, 128], BF16, tag="wfc", name=f"wfc{m}")
                        nc.sync.dma_start(wfc_m[:], d_wfc[m].rearrange("p (c n) -> p c n", c=EC))
                        ps = gpsum.tile([128, 512], F32, tag="gp", name=f"psh{m}")
                        for c in range(EC):
                            nc.tensor.matmul(ps[:, :R], wfc_m[:, c, :], z2T[:, c, :],
                                             start=(c == 0), stop=(c == EC - 1))
                        sg = wstream.tile([128, R], BF16, tag="sg", name=f"sg{m}")
                        if flags["bfc"]:
                            nc.scalar.activation(sg[:], ps[:, :R], AF.Sigmoid,
                                                 bias=bfc_s[:, m:m + 1])
                            t2 = wstream.tile([128, R], F32, tag="t2", name=f"t2{m}")
                            nc.vector.tensor_scalar(t2[:], ps[:, :R],
                                                    bfc_s[:, m:m + 1], None, ALU.add)
                            nc.vector.tensor_tensor(hT[:, m, :], t2[:], sg[:], ALU.mult)
                        else:
                            nc.scalar.activation(sg[:], ps[:, :R], AF.Sigmoid)
                            nc.vector.tensor_tensor(hT[:, m, :], ps[:, :R], sg[:], ALU.mult)
                    if "noout" in ablate:
                        nc.vector.memset(outT[:], 0.001)
                    for m in range(EC if (stage_limit >= 6 and "noout" not in ablate) else 0):
                        wout_m = wstream.tile([128, FC, 128], BF16, tag="wout", name=f"wout{m}")
                        nc.sync.dma_start(wout_m[:], d_wout[m].rearrange("p (k n) -> p k n", k=FC))
                        ps = gpsum.tile([128, 512], F32, tag="gp", name=f"pso{m}")
                        for k in range(FC):
                            nc.tensor.matmul(ps[:, :R], wout_m[:, k, :], hT[:, k, :],
                                             start=(k == 0), stop=(k == FC - 1))
                        nc.vector.tensor_tensor(outT[:, m, :], ps[:, :R], x1T[:, m, :], ALU.add)
                        if flags["bout"]:
                            nc.vector.tensor_scalar(outT[:, m, :], outT[:, m, :],
                                                    bout_s[:, m:m + 1], None, ALU.add)

                nc.sync.dma_start(d_out.rearrange("p (c n) -> p c n", c=EC), outT[:])

        if loop_n > 1:
            with tc.For_i(0, loop_n, 1):
                emit_body(0)
        else:
            for _ri in range(repeat):
                emit_body(_ri)

    nc.finalize()
    return nc


# ----------------------------------------------------------------------------
# host side: input prep / output assembly
# ----------------------------------------------------------------------------

def prepare_in_maps(cfg: Cfg, pairs, jmax, flags, inputs):
    """Build per-core input maps. Returns (in_maps, percore_blocks)."""
    x = np.asarray(inputs["x"], np.float32)
    w_qkv = np.asarray(inputs["w_qkv"], np.float32)
    b_qkv = np.asarray(inputs["b_qkv"], np.float32)
    w_proj = np.asarray(inputs["w_proj"], np.float32)
    b_proj = np.asarray(inputs["b_proj"], np.float32)
    w_fc = np.asarray(inputs["w_fc"], np.float32)
    b_fc = np.asarray(inputs["b_fc"], np.float32)
    w_out = np.asarray(inputs["w_out"], np.float32)
    b_out = np.asarray(inputs["b_out"], np.float32)
    ln1_s = np.asarray(inputs["ln1_scale"], np.float32)
    ln1_b = np.asarray(inputs["ln1_bias"], np.float32)
    ln2_s = np.asarray(inputs["ln2_scale"], np.float32)
    ln2_b = np.asarray(inputs["ln2_bias"], np.float32)
    mask_len = np.asarray(inputs["mask_len"]).astype(np.int64)

    E, L, H, D, BS = cfg.E, cfg.L, cfg.H, cfg.D, cfg.BS
    EC, FC = cfg.EC, cfg.FC

    # split qkv columns: col = h*3D + {0..D-1:q, D..2D-1:k, 2D..3D-1:v}
    wsplit = w_qkv.reshape(E, H, 3 * D)
    wq = np.ascontiguousarray(wsplit[:, :, 0:D].reshape(E, E))
    wk = np.ascontiguousarray(wsplit[:, :, D:2 * D].reshape(E, E))
    wv = np.ascontiguousarray(wsplit[:, :, 2 * D:3 * D].reshape(E, E))
    bsplit = b_qkv.reshape(H, 3 * D)
    bq = np.ascontiguousarray(bsplit[:, 0:D].reshape(E))
    bk = np.ascontiguousarray(bsplit[:, D:2 * D].reshape(E))
    bv = np.ascontiguousarray(bsplit[:, 2 * D:3 * D].reshape(E))

    def chunked_w(w):  # [E, N] -> partition-major [128, EC*N] bf16
        n = w.shape[1]
        return np.ascontiguousarray(
            w.reshape(EC, 128, n).transpose(1, 0, 2).reshape(128, EC * n)).astype(BF16NP)

    def col_f32(v):    # [E or FF] -> [128, C]
        return np.ascontiguousarray(v.reshape(-1, 128).T).astype(np.float32)

    wq_c, wk_c, wv_c, wp_c = (chunked_w(w) for w in (wq, wk, wv, w_proj))
    wfc_c = np.ascontiguousarray(
        w_fc.reshape(EC, 128, FC, 128).transpose(2, 1, 0, 3).reshape(FC, 128, EC * 128)
    ).astype(BF16NP)
    wout_c = np.ascontiguousarray(
        w_out.reshape(FC, 128, EC, 128).transpose(2, 1, 0, 3).reshape(EC, 128, FC * 128)
    ).astype(BF16NP)
    lnp = np.ascontiguousarray(np.stack(
        [col_f32(ln1_s), col_f32(ln1_b), col_f32(ln2_s), col_f32(ln2_b)]
    ).transpose(1, 0, 2))

    KS = slot_kcs(cfg, pairs, jmax)

    def core_attnmask(blocks):
        """[sum(KS), 128, BS] plain causal masks for this core's blocks."""
        parts = []
        pi = np.arange(128)[:, None]
        qi = np.arange(BS)[None, :]
        for s, j in enumerate(blocks):
            for kchunk in range(KS[s]):
                parts.append((kchunk * 128 + pi) <= (j * BS + qi))
        if not parts:
            return np.zeros((0, 128, BS), BF16NP)
        return np.stack(parts).astype(BF16NP)

    shared = dict(
        wq=wq_c, wk=wk_c, wv=wv_c, wproj=wp_c, wfc=wfc_c, wout=wout_c,
        bq=col_f32(bq), bk=col_f32(bk), bv=bv.reshape(1, E).astype(BF16NP),
        bproj=col_f32(b_proj), bfc=col_f32(b_fc), bout=col_f32(b_out),
        lnp=lnp,
    )

    in_maps = []
    percore_blocks = []
    for c in range(cfg.n_cores):
        b = c // cfg.NPOS
        g = c % cfg.NPOS
        blocks = pairs[g]
        percore_blocks.append((b, blocks))
        xT = x[b].T  # [E, L]
        own_cols = np.concatenate(
            [np.arange(j * BS, (j + 1) * BS) for j in blocks])
        sel = (own_cols < mask_len[b]).astype(BF16NP)
        selb = np.broadcast_to(sel[None, :], (128, cfg.R))
        E_, L_ = xT.shape
        EC_ = E_ // 128
        m = dict(shared)
        m["xT_full"] = np.ascontiguousarray(
            xT.reshape(EC_, 128, L_).transpose(1, 0, 2).reshape(128, EC_ * L_)).astype(BF16NP)
        xo = xT[:, own_cols]
        m["xT_own"] = np.ascontiguousarray(
            xo.reshape(EC_, 128, -1).transpose(1, 0, 2).reshape(128, -1)).astype(np.float32)
        m["selb"] = np.ascontiguousarray(selb)
        m["attnmask"] = core_attnmask(blocks)
        in_maps.append(m)
    return in_maps, percore_blocks


def assemble_output(cfg: Cfg, results, percore_blocks):
    out = np.zeros((cfg.B, cfg.L, cfg.E), np.float32)
    for c, res in enumerate(results):
        b, blocks = percore_blocks[c]
        oT = res["outT"].reshape(128, cfg.EC, cfg.R).transpose(1, 0, 2).reshape(cfg.E, cfg.R)
        for s, j in enumerate(blocks):
            out[b, j * cfg.BS:(j + 1) * cfg.BS, :] = oT[:, s * cfg.BS:(s + 1) * cfg.BS].T
    return out


def make_flags(inputs):
    def nz(name):
        return bool(np.any(np.asarray(inputs[name]) != 0))
    return dict(
        bq=nz("b_qkv"), bk=nz("b_qkv"), bv=nz("b_qkv"),
        bproj=nz("b_proj"), bfc=nz("b_fc"), bout=nz("b_out"),
        ln1aff=bool(np.any(np.asarray(inputs["ln1_scale"]) != 1)
                    or np.any(np.asarray(inputs["ln1_bias"]) != 0)),
        ln2aff=bool(np.any(np.asarray(inputs["ln2_scale"]) != 1)
                    or np.any(np.asarray(inputs["ln2_bias"]) != 0)),
    )


_cached = {}


def kernel(**inputs) -> np.ndarray:
    cfg = Cfg()
    mask_len = np.asarray(inputs["mask_len"]).astype(np.int64)
    pairs, jmax = plan_blocks(cfg, mask_len)
    flags = make_flags(inputs)
    key = (tuple(map(tuple, pairs)), jmax, tuple(sorted(flags.items())))
    if key not in _cached:
        _cached[key] = build_program(cfg, pairs, jmax, flags)
    nc = _cached[key]
    in_maps, percore_blocks = prepare_in_maps(cfg, pairs, jmax, flags, inputs)
    r = run_bass_kernel_spmd(nc, in_maps, core_ids=list(range(cfg.n_cores)))
    return assemble_output(cfg, r.results, percore_blocks)


if __name__ == "__main__":
    pass

